# revision 14
# baseline (speedup 1.0000x reference)
"""GAT layer (nn_GATLayer_28106265985525) on 8 Trainium2 NeuronCores.

Batch-parallel: core b computes graph b (bs=8). Scores are built in
transposed [j, i] layout so no per-block PE transposes are needed.

Key algebra: softmax_j is invariant to any positive per-row (i) factor, and
exp(lrelu(s)) == max(e^s, e^{0.2 s}).  With s = fs_i + fd_j + ab:
  E[i,j]           = adj * max(e^s, e^{0.2s})
  E * e^{-0.2 fs_i} = adj * max(U_i * w_j, q_j)      (row-factor cancels)
where U_i = e^{0.8 fs_i}, w_j = e^{fd_j+ab}, q_j = e^{0.2(fd_j+ab)}.
So the inner loop needs NO exp at all: per 128-j block only
  t1 = (U * w_j) max q_j     -- one 4x-mode TensorScalar on DVE (~0.6us)
  M  = t1 * adjT             -- one scalar_tensor_tensor split DVE/Pool
  out_ps[u] += M[:,u]^T @ nodesE ; Z[u] += M[:,u]^T @ 1   -- PE
U is a [128, N] broadcast tensor built once in setup; w/q are per-partition
pointer scalars.  fs/fd come from parameter-folded projections
(c1 = a1 @ W, etc.) so they are computed straight from x^T.  Final
normalization multiplies by 1/Z (reciprocal once over [128,16]).
"""

import numpy as np
from contextlib import ExitStack

N = 2048
FIN = 256
F = 128
BS = 8
TB = N // 128  # 16 row blocks
SPLIT = 832    # W2 (mask multiply) columns done on DVE; rest on Pool

_cache = {}


def _build(reps=1):
    import concourse.bass as bass
    import concourse.tile as tile
    from concourse import mybir, bacc, library_config

    f32, f16, u8 = mybir.dt.float32, mybir.dt.float16, mybir.dt.uint8
    A = mybir.AluOpType
    AF = mybir.ActivationFunctionType

    nc = bacc.Bacc("TRN2", target_bir_lowering=False, debug=False)
    xt_d = nc.declare_dram_parameter("xt", [128, 4096], f16, isOutput=False)
    adjT_d = nc.declare_dram_parameter("adjT", [N, N], u8, isOutput=False)
    wt_d = nc.declare_dram_parameter("wt", [128, FIN], f16, isOutput=False)
    wb_d = nc.declare_dram_parameter("wb", [128, 1], f32, isOutput=False)
    cb8_d = nc.declare_dram_parameter("cb8", [128, FIN], f16, isOutput=False)
    c2_d = nc.declare_dram_parameter("c2", [128, 2], f16, isOutput=False)
    bias_d = nc.declare_dram_parameter("biasc", [128, 4], f32, isOutput=False)
    idf16_d = nc.declare_dram_parameter("idf16", [128, 128], f16, isOutput=False)
    out_d = nc.declare_dram_parameter("out", [N, F], f32, isOutput=True)

    with tile.TileContext(nc) as tc, ExitStack() as ctx:
        consts = ctx.enter_context(tc.tile_pool(name="consts", bufs=1))
        adjp = ctx.enter_context(tc.tile_pool(name="adjp", bufs=6))
        t1p = ctx.enter_context(tc.tile_pool(name="t1p", bufs=2))
        mp = ctx.enter_context(tc.tile_pool(name="mp", bufs=3))
        ps_out = ctx.enter_context(tc.tile_pool(name="ps_out", bufs=1, space="PSUM"))
        ps_set = ctx.enter_context(tc.tile_pool(name="ps_set", bufs=2, space="PSUM"))
        ps_e = ctx.enter_context(tc.tile_pool(name="ps_e", bufs=1, space="PSUM"))

        # gpsimd standard library provides InstTensorTensor (mask multiply)
        nc.gpsimd.load_library(library_config.standard)

        # ---- small consts ----
        bias_sb = consts.tile([128, 4], f32)
        nc.sync.dma_start(bias_sb[:], bias_d[:, :])
        wt_sb = consts.tile([128, FIN], f16)      # [p, (c o)]
        nc.sync.dma_start(wt_sb[:], wt_d[:, :])
        cb8_sb = consts.tile([128, FIN], f16)     # [p, (c m)]
        nc.sync.dma_start(cb8_sb[:], cb8_d[:, :])
        c2_sb = consts.tile([128, 2], f16)
        nc.sync.dma_start(c2_sb[:], c2_d[:, :])
        wb_sb = consts.tile([128, 1], f32)
        nc.sync.dma_start(wb_sb[:], wb_d[:, :])
        idf16 = consts.tile([128, 128], f16)
        nc.sync.dma_start(idf16[:], idf16_d[:, :])
        ones_col = consts.tile([128, 1], f16)
        nc.vector.memset(ones_col[:], 1.0)
        warm = consts.tile([128, 1], f32)
        # preload the Exp activation table while DMAs stream
        nc.scalar.activation(warm[:], bias_sb[:, 3:4], AF.Exp)

        wt_v = wt_sb[:].rearrange("p (c o) -> p c o", c=2)
        cb8_v = cb8_sb[:].rearrange("p (c m) -> p c m", c=2)

        # ---- xt in 4 chunks: [p, g, c, 512] ----
        xt_sb = consts.tile([128, 4096], f16)
        xt_v = xt_sb[:].rearrange("p (g c n) -> p g c n", g=4, c=2)
        for g in range(4):
            eng = [nc.scalar, nc.scalar, nc.gpsimd, nc.gpsimd][g]
            eng.dma_start(
                xt_sb[:, g * 1024:(g + 1) * 1024], xt_d[:, g * 1024:(g + 1) * 1024]
            )

        # ---- persistent PSUM ----
        out_b = [
            ps_out.tile([128, 512], f32, name=f"ob{k}") for k in range(4)
        ]
        misc_ps = ps_out.tile([128, 32], f32)  # cols 0:16 Z, 16:32 fd

        nT16 = consts.tile([128, N], f16)   # nodes^T [o, n]
        U = consts.tile([128, N], f16)      # e^{0.8 fs_i}, broadcast rows
        wq = consts.tile([128, 32], f32)    # cols 0:16 w, 16:32 q
        nE = consts.tile([128, TB * 128], f16)  # nodes [j, o] per block
        nE_v = nE[:].rearrange("p (t o) -> p t o", o=128)

        # ---- U path first (gates the W1 stream) ----
        for g in range(4):
            u_ps = ps_set.tile([128, 512], f32, tag="s")
            for c in range(2):
                nc.tensor.matmul(
                    u_ps[:], cb8_v[:, c, :], xt_v[:, g, c, :],
                    start=(c == 0), stop=(c == 1),
                )
            nc.scalar.activation(
                U[:, g * 512:(g + 1) * 512], u_ps[:],
                AF.Exp, bias=bias_sb[:, 0:1], scale=1.0,
            )

        # ---- fd: misc_ps[:, 16+t] = sum_f c2[f] * xT[f, j-block t] ----
        for t in range(TB):
            g, sub = t // 4, t % 4
            for c in range(2):
                nc.tensor.matmul(
                    misc_ps[:, 16 + t:17 + t],
                    xt_v[:, g, c, sub * 128:(sub + 1) * 128],
                    c2_sb[:, c:c + 1],
                    start=(c == 0), stop=(c == 1),
                )
        nc.scalar.activation(
            wq[:, 0:16], misc_ps[:, 16:32], AF.Exp,
            bias=bias_sb[:, 1:2], scale=1.0,
        )
        nc.scalar.activation(
            wq[:, 16:32], misc_ps[:, 16:32], AF.Exp,
            bias=bias_sb[:, 2:3], scale=0.2,
        )

        # ---- nodes^T = W @ x^T + b (evac on Pool), then nE via PE transpose ----
        for g in range(4):
            n_ps = ps_set.tile([128, 512], f32, tag="s")
            for c in range(2):
                nc.tensor.matmul(
                    n_ps[:], wt_v[:, c, :], xt_v[:, g, c, :],
                    start=(c == 0), stop=(c == 1),
                )
            nc.vector.tensor_scalar(
                nT16[:, g * 512:(g + 1) * 512], n_ps[:], wb_sb[:], None, A.add
            )
        for g in range(4):
            e_ps = ps_e.tile([128, 512], f16, tag="e")
            for k in range(4):
                t = g * 4 + k
                nc.tensor.transpose(
                    e_ps[:, k * 128:(k + 1) * 128],
                    nT16[:, t * 128:(t + 1) * 128],
                    idf16[:],
                )
            nc.scalar.activation(
                nE[:, g * 512:(g + 1) * 512], e_ps[:], AF.Identity
            )

        # ---- main loop ----
        for rep in range(reps):
            for t in range(TB):
                adj_t = adjp.tile([128, N], u8, tag="adj")
                nc.sync.dma_start(adj_t[:], adjT_d[t * 128:(t + 1) * 128, :])

                t1 = t1p.tile([128, N], f16, tag="t1")
                nc.vector.tensor_scalar(
                    t1[:], U[:], wq[:, t:t + 1], wq[:, 16 + t:17 + t],
                    A.mult, A.max,
                )
                M = mp.tile([128, N], f16, tag="M")
                nc.vector.tensor_tensor(
                    M[:, 0:SPLIT], t1[:, 0:SPLIT], adj_t[:, 0:SPLIT], A.mult
                )
                nc.gpsimd.tensor_tensor(
                    M[:, SPLIT:N], t1[:, SPLIT:N], adj_t[:, SPLIT:N], A.mult
                )
                for u in range(TB):
                    # start=True clears has_written bits for the WHOLE bank,
                    # so only the first chunk per bank may clear; later
                    # chunks land on cleared bits -> first write overwrites.
                    nc.tensor.matmul(
                        out_b[u // 4][:, (u % 4) * 128:(u % 4 + 1) * 128],
                        M[:, u * 128:(u + 1) * 128],
                        nE_v[:, t, :],
                        start=(t == 0 and u % 4 == 0), stop=(t == TB - 1),
                    )
                    nc.tensor.matmul(
                        misc_ps[:, u:u + 1],
                        M[:, u * 128:(u + 1) * 128],
                        ones_col[:],
                        start=(t == 0 and u == 0), stop=(t == TB - 1),
                    )

            # ---- normalize + write out ----
            rc = consts.tile([128, 16], f32, tag="rc")
            nc.vector.reciprocal(rc[:], misc_ps[:, 0:16])
            osb = consts.tile([128, TB * 128], f32, tag="osb")
            for u in range(TB):
                src = out_b[u // 4][:, (u % 4) * 128:(u % 4 + 1) * 128]
                dst = osb[:, u * 128:(u + 1) * 128]
                if u % 2 == 0:
                    nc.vector.tensor_scalar(dst, src, rc[:, u:u + 1], None, A.mult)
                else:
                    nc.scalar.activation(dst, src, AF.Copy, bias=0.0, scale=rc[:, u:u + 1])
            out_v = out_d[:, :].rearrange("(t p) o -> p t o", p=128)
            osb_v = osb[:].rearrange("p (t o) -> p t o", o=128)
            for q in range(4):
                eng = [nc.sync, nc.sync, nc.scalar, nc.gpsimd][q]
                eng.dma_start(
                    out_v[:, q * 4:(q + 1) * 4, :], osb_v[:, q * 4:(q + 1) * 4, :]
                )

    nc.compile()
    return nc


def make_in_maps(inputs, adjs, W_w, W_b, a_w, a_b):
    inputs = np.asarray(inputs, dtype=np.float32)
    adjs = np.asarray(adjs)
    W_w = np.asarray(W_w, dtype=np.float32)
    W_b = np.asarray(W_b, dtype=np.float32)
    a_w = np.asarray(a_w, dtype=np.float32)
    ab = float(np.asarray(a_b, dtype=np.float32).reshape(()))

    # xt[p, g, c, n'] = x^T[c*128+p, g*512+n']
    def pack_xt(xb):
        xT = xb.T.astype(np.float16)                      # [256, 2048]
        v = xT.reshape(2, 128, 4, 512)                    # [c, p, g, n']
        return np.ascontiguousarray(
            v.transpose(1, 2, 0, 3).reshape(128, 4096))   # [p, g, c, n']

    # wt[p, c, o] = W_w[o, c*128+p]
    wt = np.ascontiguousarray(
        W_w.T.reshape(2, 128, 128).transpose(1, 0, 2).reshape(128, 256)
    ).astype(np.float16)
    wb = np.ascontiguousarray(W_b.reshape(128, 1)).astype(np.float32)

    # parameter-folded projections
    a1 = a_w[0, :F]
    a2 = a_w[0, F:]
    c1 = a1 @ W_w          # [256]
    c2 = a2 @ W_w          # [256]
    d1 = float(a1 @ W_b)
    d2 = float(a2 @ W_b)
    # cb8[p, c, m] = 0.8*c1[c*128+p] (broadcast over m)
    cb8 = np.ascontiguousarray(
        np.broadcast_to((0.8 * c1).reshape(2, 128, 1).transpose(1, 0, 2),
                        (128, 2, 128)).reshape(128, 256)
    ).astype(np.float16)
    c2p = np.ascontiguousarray(
        c2.reshape(2, 128).T
    ).astype(np.float16)   # [p, c]
    biasc = np.ascontiguousarray(
        np.broadcast_to(
            np.array([0.8 * d1, d2 + ab, 0.2 * (d2 + ab), 0.0],
                     dtype=np.float32),
            (128, 4),
        )
    )
    idf16 = np.eye(128, dtype=np.float16)

    maps = []
    for b in range(BS):
        maps.append({
            "xt": pack_xt(inputs[b]),
            "adjT": np.ascontiguousarray(adjs[b].T).astype(np.uint8),
            "wt": wt,
            "wb": wb,
            "cb8": cb8,
            "c2": c2p,
            "biasc": biasc,
            "idf16": idf16,
        })
    return maps


def kernel(inputs, adjs, W_w, W_b, a_w, a_b):
    from concourse.bass_utils import run_bass_kernel_spmd

    if "nc" not in _cache:
        _cache["nc"] = _build()
    nc = _cache["nc"]

    in_maps = make_in_maps(inputs, adjs, W_w, W_b, a_w, a_b)
    try:
        res = run_bass_kernel_spmd(nc, in_maps, core_ids=list(range(BS)))
    except Exception:
        # transient NRT_EXEC_UNIT_UNRECOVERABLE etc. -- retry once
        res = run_bass_kernel_spmd(nc, in_maps, core_ids=list(range(BS)))
    out = np.stack([res.results[b]["out"] for b in range(BS)], axis=0)
    return out.astype(np.float32)


# revision 18
# speedup vs baseline: 1.1877x; 1.1877x over previous
"""GAT layer (nn_GATLayer_28106265985525) on 8 Trainium2 NeuronCores.

Batch-parallel: core b computes graph b (bs=8). Scores are built in
transposed [j, i] layout so no per-block PE transposes are needed.

Key algebra: softmax_j is invariant to any positive per-row (i) factor, and
exp(lrelu(s)) == max(e^s, e^{0.2 s}).  With s = fs_i + fd_j + ab:
  E[i,j]            = adj * max(e^s, e^{0.2s})
  E * e^{-0.2 fs_i} = adj * max(U_i * w_j, q_j)      (row-factor cancels)
where U_i = e^{0.8 fs_i}, w_j = e^{fd_j+ab}, q_j = e^{0.2(fd_j+ab)}.
So the inner loop needs NO exp at all: per 128-j block only
  t1 = (U * w_j) max q_j     -- one 4x-mode TensorScalar on DVE (~0.6us)
  M  = t1 * adjT             -- mask multiply, split DVE (f16 mask, 2x
                                TensorTensor) / Pool gpsimd (u8 mask)
  out_ps[u] += M[:,u]^T @ nodesE ; Z[u] += M[:,u]^T @ 1   -- PE
U is a [128, N] broadcast tensor built once in setup; w/q are per-partition
pointer scalars.  fs/fd come from parameter-folded projections
(c1 = a1 @ W etc.) so they are computed straight from x^T.  Final
normalization multiplies by 1/Z (reciprocal over [128,4] quarters).
PSUM note: matmul start=True clears has_written bits for the whole bank,
so only the first accumulation chain per bank clears; later chunks rely on
"overwrite where bit unset" first-write semantics.
"""

import numpy as np
from contextlib import ExitStack

N = 2048
FIN = 256
F = 128
BS = 8
TB = N // 128  # 16 row blocks
SPLIT = 832    # mask-multiply columns on DVE (f16 mask); rest on Pool (u8)

_cache = {}


def _build(reps=1):
    import concourse.bass as bass
    import concourse.tile as tile
    from concourse import mybir, bacc, library_config

    f32, f16, u8 = mybir.dt.float32, mybir.dt.float16, mybir.dt.uint8
    A = mybir.AluOpType
    AF = mybir.ActivationFunctionType

    nc = bacc.Bacc("TRN2", target_bir_lowering=False, debug=False)
    xt_d = nc.declare_dram_parameter("xt", [128, 4096], f16, isOutput=False)
    adjf_d = nc.declare_dram_parameter("adjf", [N, SPLIT], f16, isOutput=False)
    adju_d = nc.declare_dram_parameter("adju", [N, N - SPLIT], u8, isOutput=False)
    wt_d = nc.declare_dram_parameter("wt", [128, FIN], f16, isOutput=False)
    wb_d = nc.declare_dram_parameter("wb", [128, 1], f32, isOutput=False)
    cb8_d = nc.declare_dram_parameter("cb8", [128, FIN], f16, isOutput=False)
    c2_d = nc.declare_dram_parameter("c2", [128, 2], f16, isOutput=False)
    bias_d = nc.declare_dram_parameter("biasc", [128, 4], f32, isOutput=False)
    idf16_d = nc.declare_dram_parameter("idf16", [128, 128], f16, isOutput=False)
    out_d = nc.declare_dram_parameter("out", [N, F], f32, isOutput=True)

    with tile.TileContext(nc) as tc, ExitStack() as ctx:
        consts = ctx.enter_context(tc.tile_pool(name="consts", bufs=1))
        adjp = ctx.enter_context(tc.tile_pool(name="adjp", bufs=6))
        t1p = ctx.enter_context(tc.tile_pool(name="t1p", bufs=3))
        mp = ctx.enter_context(tc.tile_pool(name="mp", bufs=4))
        ps_out = ctx.enter_context(tc.tile_pool(name="ps_out", bufs=1, space="PSUM"))
        ps_set = ctx.enter_context(tc.tile_pool(name="ps_set", bufs=2, space="PSUM"))
        ps_e = ctx.enter_context(tc.tile_pool(name="ps_e", bufs=1, space="PSUM"))

        # gpsimd standard library provides InstTensorTensor (mask multiply)
        nc.gpsimd.load_library(library_config.standard)

        # ---- warm the Exp table from a memset input (no DMA dependency) ----
        warm_in = consts.tile([128, 1], f32)
        nc.vector.memset(warm_in[:], 0.0)
        warm = consts.tile([128, 1], f32)
        nc.scalar.activation(warm[:], warm_in[:], AF.Exp)
        ones_col = consts.tile([128, 1], f16)
        nc.vector.memset(ones_col[:], 1.0)

        # ---- consts: order matters (SP in-order; earliest needed first) ----
        cb8_sb = consts.tile([128, FIN], f16)     # [p, (c m)]
        nc.sync.dma_start(cb8_sb[:], cb8_d[:, :])
        bias_sb = consts.tile([128, 4], f32)
        nc.sync.dma_start(bias_sb[:], bias_d[:, :])
        c2_sb = consts.tile([128, 2], f16)
        nc.sync.dma_start(c2_sb[:], c2_d[:, :])
        wt_sb = consts.tile([128, FIN], f16)      # [p, (c o)]
        nc.sync.dma_start(wt_sb[:], wt_d[:, :])
        wb_sb = consts.tile([128, 1], f32)
        nc.sync.dma_start(wb_sb[:], wb_d[:, :])
        idf16 = consts.tile([128, 128], f16)
        nc.sync.dma_start(idf16[:], idf16_d[:, :])

        wt_v = wt_sb[:].rearrange("p (c o) -> p c o", c=2)
        cb8_v = cb8_sb[:].rearrange("p (c m) -> p c m", c=2)

        # ---- xt in 4 chunks: [p, g, c, 512]; ACT + Pool queues ----
        xt_sb = consts.tile([128, 4096], f16)
        xt_v = xt_sb[:].rearrange("p (g c n) -> p g c n", g=4, c=2)
        for g in range(4):
            eng = [nc.scalar, nc.scalar, nc.gpsimd, nc.gpsimd][g]
            eng.dma_start(
                xt_sb[:, g * 1024:(g + 1) * 1024], xt_d[:, g * 1024:(g + 1) * 1024]
            )

        # ---- persistent PSUM ----
        out_b = [
            ps_out.tile([128, 512], f32, name=f"ob{k}") for k in range(4)
        ]
        misc_ps = ps_out.tile([128, 32], f32)  # cols 0:16 Z, 16:32 fd

        nT16 = consts.tile([128, N], f16)   # nodes^T [o, n]
        U = consts.tile([128, N], f16)      # e^{0.8 fs_i}, broadcast rows
        wq = consts.tile([128, 32], f32)    # cols 0:16 w, 16:32 q
        nE = consts.tile([128, TB * 128], f16)  # nodes [j, o] per block
        nE_v = nE[:].rearrange("p (t o) -> p t o", o=128)

        # ---- U path first (gates the W1 stream) ----
        for g in range(4):
            u_ps = ps_set.tile([128, 512], f32, tag="s")
            for c in range(2):
                nc.tensor.matmul(
                    u_ps[:], cb8_v[:, c, :], xt_v[:, g, c, :],
                    start=(c == 0), stop=(c == 1),
                )
            nc.scalar.activation(
                U[:, g * 512:(g + 1) * 512], u_ps[:],
                AF.Exp, bias=bias_sb[:, 0:1], scale=1.0,
            )

        # ---- fd: misc_ps[:, 16+t] = sum_f c2[f] * xT[f, j-block t] ----
        for t in range(TB):
            g, sub = t // 4, t % 4
            for c in range(2):
                nc.tensor.matmul(
                    misc_ps[:, 16 + t:17 + t],
                    xt_v[:, g, c, sub * 128:(sub + 1) * 128],
                    c2_sb[:, c:c + 1],
                    start=(c == 0), stop=(c == 1),
                )
        nc.scalar.activation(
            wq[:, 0:16], misc_ps[:, 16:32], AF.Exp,
            bias=bias_sb[:, 1:2], scale=1.0,
        )
        nc.scalar.activation(
            wq[:, 16:32], misc_ps[:, 16:32], AF.Exp,
            bias=bias_sb[:, 2:3], scale=0.2,
        )

        # ---- nodes^T = W @ x^T + b (evac on ACT), then nE via PE transpose ----
        for g in range(4):
            n_ps = ps_set.tile([128, 512], f32, tag="s")
            for c in range(2):
                nc.tensor.matmul(
                    n_ps[:], wt_v[:, c, :], xt_v[:, g, c, :],
                    start=(c == 0), stop=(c == 1),
                )
            nc.scalar.activation(
                nT16[:, g * 512:(g + 1) * 512], n_ps[:],
                AF.Identity, bias=wb_sb[:], scale=1.0,
            )
        for g in range(4):
            e_ps = ps_e.tile([128, 512], f16, tag="e")
            for k in range(4):
                t = g * 4 + k
                nc.tensor.transpose(
                    e_ps[:, k * 128:(k + 1) * 128],
                    nT16[:, t * 128:(t + 1) * 128],
                    idf16[:],
                )
            nc.scalar.activation(
                nE[:, g * 512:(g + 1) * 512], e_ps[:], AF.Identity
            )

        # ---- main loop ----
        for rep in range(reps):
            for t in range(TB):
                adjf_t = adjp.tile([128, SPLIT], f16, tag="adjf")
                nc.sync.dma_start(adjf_t[:], adjf_d[t * 128:(t + 1) * 128, :])
                adju_t = adjp.tile([128, N - SPLIT], u8, tag="adju")
                nc.sync.dma_start(adju_t[:], adju_d[t * 128:(t + 1) * 128, :])

                t1 = t1p.tile([128, N], f16, tag="t1")
                nc.vector.tensor_scalar(
                    t1[:], U[:], wq[:, t:t + 1], wq[:, 16 + t:17 + t],
                    A.mult, A.max,
                )
                M = mp.tile([128, N], f16, tag="M")
                nc.vector.tensor_tensor(
                    M[:, 0:SPLIT], t1[:, 0:SPLIT], adjf_t[:], A.mult
                )
                nc.gpsimd.tensor_tensor(
                    M[:, SPLIT:N], t1[:, SPLIT:N], adju_t[:], A.mult
                )
                for u in range(TB):
                    # start=True clears has_written bits for the WHOLE bank:
                    # only the first chunk per bank clears; later chunks land
                    # on cleared bits -> first write overwrites.
                    nc.tensor.matmul(
                        out_b[u // 4][:, (u % 4) * 128:(u % 4 + 1) * 128],
                        M[:, u * 128:(u + 1) * 128],
                        nE_v[:, t, :],
                        start=(t == 0 and u % 4 == 0), stop=(t == TB - 1),
                    )
                    nc.tensor.matmul(
                        misc_ps[:, u:u + 1],
                        M[:, u * 128:(u + 1) * 128],
                        ones_col[:],
                        start=(t == 0 and u == 0), stop=(t == TB - 1),
                    )

            # ---- normalize + write out, pipelined per 4-chunk quarter ----
            rc = consts.tile([128, 16], f32, tag="rc")
            osb = consts.tile([128, TB * 128], f32, tag="osb")
            out_v = out_d[:, :].rearrange("(t p) o -> p t o", p=128)
            osb_v = osb[:].rearrange("p (t o) -> p t o", o=128)
            for qq in range(4):
                nc.vector.reciprocal(
                    rc[:, qq * 4:(qq + 1) * 4], misc_ps[:, qq * 4:(qq + 1) * 4]
                )
                for u in range(qq * 4, qq * 4 + 4):
                    src = out_b[u // 4][:, (u % 4) * 128:(u % 4 + 1) * 128]
                    dst = osb[:, u * 128:(u + 1) * 128]
                    if u % 2 == 0:
                        nc.vector.tensor_scalar(
                            dst, src, rc[:, u:u + 1], None, A.mult
                        )
                    else:
                        nc.scalar.activation(
                            dst, src, AF.Copy, bias=0.0, scale=rc[:, u:u + 1]
                        )
                eng = [nc.sync, nc.scalar, nc.gpsimd, nc.sync][qq]
                eng.dma_start(
                    out_v[:, qq * 4:(qq + 1) * 4, :],
                    osb_v[:, qq * 4:(qq + 1) * 4, :],
                )

    nc.compile()
    return nc


def make_in_maps(inputs, adjs, W_w, W_b, a_w, a_b):
    inputs = np.asarray(inputs, dtype=np.float32)
    adjs = np.asarray(adjs)
    W_w = np.asarray(W_w, dtype=np.float32)
    W_b = np.asarray(W_b, dtype=np.float32)
    a_w = np.asarray(a_w, dtype=np.float32)
    ab = float(np.asarray(a_b, dtype=np.float32).reshape(()))

    # xt[p, g, c, n'] = x^T[c*128+p, g*512+n']
    def pack_xt(xb):
        xT = xb.T.astype(np.float16)                      # [256, 2048]
        v = xT.reshape(2, 128, 4, 512)                    # [c, p, g, n']
        return np.ascontiguousarray(
            v.transpose(1, 2, 0, 3).reshape(128, 4096))   # [p, g, c, n']

    # wt[p, c, o] = W_w[o, c*128+p]
    wt = np.ascontiguousarray(
        W_w.T.reshape(2, 128, 128).transpose(1, 0, 2).reshape(128, 256)
    ).astype(np.float16)
    wb = np.ascontiguousarray(W_b.reshape(128, 1)).astype(np.float32)

    # parameter-folded projections
    a1 = a_w[0, :F]
    a2 = a_w[0, F:]
    c1 = a1 @ W_w          # [256]
    c2 = a2 @ W_w          # [256]
    d1 = float(a1 @ W_b)
    d2 = float(a2 @ W_b)
    # cb8[p, c, m] = 0.8*c1[c*128+p] (broadcast over m)
    cb8 = np.ascontiguousarray(
        np.broadcast_to((0.8 * c1).reshape(2, 128, 1).transpose(1, 0, 2),
                        (128, 2, 128)).reshape(128, 256)
    ).astype(np.float16)
    c2p = np.ascontiguousarray(
        c2.reshape(2, 128).T
    ).astype(np.float16)   # [p, c]
    biasc = np.ascontiguousarray(
        np.broadcast_to(
            np.array([0.8 * d1, d2 + ab, 0.2 * (d2 + ab), 0.0],
                     dtype=np.float32),
            (128, 4),
        )
    )
    idf16 = np.eye(128, dtype=np.float16)

    maps = []
    for b in range(BS):
        adjT = adjs[b].T
        maps.append({
            "xt": pack_xt(inputs[b]),
            "adjf": np.ascontiguousarray(adjT[:, :SPLIT]).astype(np.float16),
            "adju": np.ascontiguousarray(adjT[:, SPLIT:]).astype(np.uint8),
            "wt": wt,
            "wb": wb,
            "cb8": cb8,
            "c2": c2p,
            "biasc": biasc,
            "idf16": idf16,
        })
    return maps


def kernel(inputs, adjs, W_w, W_b, a_w, a_b):
    from concourse.bass_utils import run_bass_kernel_spmd

    if "nc" not in _cache:
        _cache["nc"] = _build()
    nc = _cache["nc"]

    in_maps = make_in_maps(inputs, adjs, W_w, W_b, a_w, a_b)
    try:
        res = run_bass_kernel_spmd(nc, in_maps, core_ids=list(range(BS)))
    except Exception:
        # transient NRT_EXEC_UNIT_UNRECOVERABLE etc. -- retry once
        res = run_bass_kernel_spmd(nc, in_maps, core_ids=list(range(BS)))
    out = np.stack([res.results[b]["out"] for b in range(BS)], axis=0)
    return out.astype(np.float32)


# revision 21
# speedup vs baseline: 1.2414x; 1.0452x over previous
"""GAT layer (nn_GATLayer_28106265985525) on 8 Trainium2 NeuronCores.

Batch-parallel: core b computes graph b (bs=8). Scores are built in
transposed [j, i] layout so no per-block PE transposes are needed.

Key algebra: softmax_j is invariant to any positive per-row (i) factor, and
exp(lrelu(s)) == max(e^s, e^{0.2 s}).  With s = fs_i + fd_j + ab:
  E[i,j]            = adj * max(e^s, e^{0.2s})
  E * e^{-0.2 fs_i} = adj * max(U_i * w_j, q_j)      (row-factor cancels)
where U_i = e^{0.8 fs_i}, w_j = e^{fd_j+ab}, q_j = e^{0.2(fd_j+ab)}.
So the inner loop needs NO exp at all: per 128-j block only
  t1 = (U * w_j) max q_j     -- one 4x-mode TensorScalar on DVE (~0.6us)
  M  = t1 * adjT             -- mask multiply, split DVE (f16 mask, 2x
                                TensorTensor) / Pool gpsimd (u8 mask)
  out_ps[u] += M[:,u]^T @ nodesE ; Z[u] += M[:,u]^T @ 1   -- PE
U is a [128, N] broadcast tensor built once in setup; w/q are per-partition
pointer scalars.  fs/fd come from parameter-folded projections
(c1 = a1 @ W etc.) so they are computed straight from x^T.  Final
normalization multiplies by 1/Z (reciprocal over [128,4] quarters).
PSUM note: matmul start=True clears has_written bits for the whole bank,
so only the first accumulation chain per bank clears; later chunks rely on
"overwrite where bit unset" first-write semantics.
"""

import numpy as np
from contextlib import ExitStack

N = 2048
FIN = 256
F = 128
BS = 8
TB = N // 128  # 16 row blocks
SPLIT = 832    # mask-multiply columns on DVE (f16 mask); rest on Pool (u8)

_cache = {}


def _build(reps=1):
    import concourse.bass as bass
    import concourse.tile as tile
    from concourse import mybir, bacc, library_config

    f32, f16, u8 = mybir.dt.float32, mybir.dt.float16, mybir.dt.uint8
    A = mybir.AluOpType
    AF = mybir.ActivationFunctionType

    nc = bacc.Bacc("TRN2", target_bir_lowering=False, debug=False)
    xt_d = nc.declare_dram_parameter("xt", [128, 4096], f16, isOutput=False)
    adjf_d = nc.declare_dram_parameter("adjf", [N, SPLIT], f16, isOutput=False)
    adju_d = nc.declare_dram_parameter("adju", [N, N - SPLIT], u8, isOutput=False)
    wt_d = nc.declare_dram_parameter("wt", [128, FIN], f16, isOutput=False)
    wb_d = nc.declare_dram_parameter("wb", [128, 1], f32, isOutput=False)
    cb8_d = nc.declare_dram_parameter("cb8", [128, FIN], f16, isOutput=False)
    c2_d = nc.declare_dram_parameter("c2", [128, 2], f16, isOutput=False)
    bias_d = nc.declare_dram_parameter("biasc", [128, 4], f32, isOutput=False)
    idf16_d = nc.declare_dram_parameter("idf16", [128, 128], f16, isOutput=False)
    out_d = nc.declare_dram_parameter("out", [N, F], f32, isOutput=True)

    with tile.TileContext(nc) as tc, ExitStack() as ctx:
        consts = ctx.enter_context(tc.tile_pool(name="consts", bufs=1))
        adjp = ctx.enter_context(tc.tile_pool(name="adjp", bufs=6))
        t1p = ctx.enter_context(tc.tile_pool(name="t1p", bufs=3))
        mp = ctx.enter_context(tc.tile_pool(name="mp", bufs=4))
        ps_out = ctx.enter_context(tc.tile_pool(name="ps_out", bufs=1, space="PSUM"))
        ps_set = ctx.enter_context(tc.tile_pool(name="ps_set", bufs=2, space="PSUM"))
        ps_e = ctx.enter_context(tc.tile_pool(name="ps_e", bufs=1, space="PSUM"))

        # gpsimd standard library provides InstTensorTensor (mask multiply)
        nc.gpsimd.load_library(library_config.standard)

        warm_in = consts.tile([128, 1], f32)
        nc.vector.memset(warm_in[:], 0.0)
        ones_col = consts.tile([128, 1], f16)
        nc.vector.memset(ones_col[:], 1.0)

        # ---- consts: order matters (SP in-order; earliest needed first) ----
        cb8_sb = consts.tile([128, FIN], f16)     # [p, (c m)]
        nc.sync.dma_start(cb8_sb[:], cb8_d[:, :])
        bias_sb = consts.tile([128, 4], f32)
        nc.sync.dma_start(bias_sb[:], bias_d[:, :])
        c2_sb = consts.tile([128, 2], f16)
        nc.sync.dma_start(c2_sb[:], c2_d[:, :])
        wt_sb = consts.tile([128, FIN], f16)      # [p, (c o)]
        nc.sync.dma_start(wt_sb[:], wt_d[:, :])
        wb_sb = consts.tile([128, 1], f32)
        nc.sync.dma_start(wb_sb[:], wb_d[:, :])
        idf16 = consts.tile([128, 128], f16)
        nc.sync.dma_start(idf16[:], idf16_d[:, :])

        wt_v = wt_sb[:].rearrange("p (c o) -> p c o", c=2)
        cb8_v = cb8_sb[:].rearrange("p (c m) -> p c m", c=2)

        # ---- xt in 4 chunks: [p, g, c, 512]; xt0 on ACT, rest on Pool ----
        xt_sb = consts.tile([128, 4096], f16)
        xt_v = xt_sb[:].rearrange("p (g c n) -> p g c n", g=4, c=2)
        for g in range(4):
            eng = [nc.scalar, nc.gpsimd, nc.gpsimd, nc.gpsimd][g]
            eng.dma_start(
                xt_sb[:, g * 1024:(g + 1) * 1024], xt_d[:, g * 1024:(g + 1) * 1024]
            )
        # warm the Exp table while xt streams (memset input, no DMA dep)
        warm = consts.tile([128, 1], f32)
        nc.scalar.activation(warm[:], warm_in[:], AF.Exp)

        # ---- persistent PSUM ----
        out_b = [
            ps_out.tile([128, 512], f32, name=f"ob{k}") for k in range(4)
        ]
        misc_ps = ps_out.tile([128, 32], f32)  # cols 0:16 Z, 16:32 fd

        nT16 = consts.tile([128, N], f16)   # nodes^T [o, n]
        U = consts.tile([128, N], f16)      # e^{0.8 fs_i}, broadcast rows
        wq = consts.tile([128, 32], f32)    # cols 0:16 w, 16:32 q
        nE = consts.tile([128, TB * 128], f16)  # nodes [j, o] per block
        nE_v = nE[:].rearrange("p (t o) -> p t o", o=128)

        # ---- U path first (gates the W1 stream) ----
        for g in range(4):
            u_ps = ps_set.tile([128, 512], f32, tag="s")
            for c in range(2):
                nc.tensor.matmul(
                    u_ps[:], cb8_v[:, c, :], xt_v[:, g, c, :],
                    start=(c == 0), stop=(c == 1),
                )
            nc.scalar.activation(
                U[:, g * 512:(g + 1) * 512], u_ps[:],
                AF.Exp, bias=bias_sb[:, 0:1], scale=1.0,
            )

        # ---- fd: misc_ps[:, 16+t] = sum_f c2[f] * xT[f, j-block t] ----
        for t in range(TB):
            g, sub = t // 4, t % 4
            for c in range(2):
                nc.tensor.matmul(
                    misc_ps[:, 16 + t:17 + t],
                    xt_v[:, g, c, sub * 128:(sub + 1) * 128],
                    c2_sb[:, c:c + 1],
                    start=(c == 0), stop=(c == 1),
                )
        nc.scalar.activation(
            wq[:, 0:16], misc_ps[:, 16:32], AF.Exp,
            bias=bias_sb[:, 1:2], scale=1.0,
        )
        nc.scalar.activation(
            wq[:, 16:32], misc_ps[:, 16:32], AF.Exp,
            bias=bias_sb[:, 2:3], scale=0.2,
        )

        # ---- nodes^T = W @ x^T + b (evac on ACT), then nE via PE transpose ----
        for g in range(4):
            n_ps = ps_set.tile([128, 512], f32, tag="s")
            for c in range(2):
                nc.tensor.matmul(
                    n_ps[:], wt_v[:, c, :], xt_v[:, g, c, :],
                    start=(c == 0), stop=(c == 1),
                )
            nc.scalar.activation(
                nT16[:, g * 512:(g + 1) * 512], n_ps[:],
                AF.Identity, bias=wb_sb[:], scale=1.0,
            )
        for g in range(4):
            e_ps = ps_e.tile([128, 512], f16, tag="e")
            for k in range(4):
                t = g * 4 + k
                nc.tensor.transpose(
                    e_ps[:, k * 128:(k + 1) * 128],
                    nT16[:, t * 128:(t + 1) * 128],
                    idf16[:],
                )
            nc.scalar.activation(
                nE[:, g * 512:(g + 1) * 512], e_ps[:], AF.Identity
            )

        # ---- main loop ----
        for rep in range(reps):
            for t in range(TB):
                adjf_t = adjp.tile([128, SPLIT], f16, tag="adjf")
                nc.sync.dma_start(adjf_t[:], adjf_d[t * 128:(t + 1) * 128, :])
                adju_t = adjp.tile([128, N - SPLIT], u8, tag="adju")
                nc.sync.dma_start(adju_t[:], adju_d[t * 128:(t + 1) * 128, :])

                t1 = t1p.tile([128, N], f16, tag="t1")
                nc.vector.tensor_scalar(
                    t1[:], U[:], wq[:, t:t + 1], wq[:, 16 + t:17 + t],
                    A.mult, A.max,
                )
                M = mp.tile([128, N], f16, tag="M")
                nc.vector.tensor_tensor(
                    M[:, 0:SPLIT], t1[:, 0:SPLIT], adjf_t[:], A.mult
                )
                nc.gpsimd.tensor_tensor(
                    M[:, SPLIT:N], t1[:, SPLIT:N], adju_t[:], A.mult
                )
                for u in range(TB):
                    # start=True clears has_written bits for the WHOLE bank:
                    # only the first chunk per bank clears; later chunks land
                    # on cleared bits -> first write overwrites.
                    nc.tensor.matmul(
                        out_b[u // 4][:, (u % 4) * 128:(u % 4 + 1) * 128],
                        M[:, u * 128:(u + 1) * 128],
                        nE_v[:, t, :],
                        start=(t == 0 and u % 4 == 0), stop=(t == TB - 1),
                    )
                    nc.tensor.matmul(
                        misc_ps[:, u:u + 1],
                        M[:, u * 128:(u + 1) * 128],
                        ones_col[:],
                        start=(t == 0 and u == 0), stop=(t == TB - 1),
                    )

            # ---- normalize + write out, pipelined per 4-chunk quarter ----
            rc = consts.tile([128, 16], f32, tag="rc")
            osb = consts.tile([128, TB * 128], f32, tag="osb")
            out_v = out_d[:, :].rearrange("(t p) o -> p t o", p=128)
            osb_v = osb[:].rearrange("p (t o) -> p t o", o=128)
            for qq in range(4):
                nc.vector.reciprocal(
                    rc[:, qq * 4:(qq + 1) * 4], misc_ps[:, qq * 4:(qq + 1) * 4]
                )
                for u in range(qq * 4, qq * 4 + 4):
                    src = out_b[u // 4][:, (u % 4) * 128:(u % 4 + 1) * 128]
                    dst = osb[:, u * 128:(u + 1) * 128]
                    if u % 2 == 0:
                        nc.vector.tensor_scalar(
                            dst, src, rc[:, u:u + 1], None, A.mult
                        )
                    else:
                        nc.scalar.activation(
                            dst, src, AF.Copy, bias=0.0, scale=rc[:, u:u + 1]
                        )
                eng = [nc.sync, nc.gpsimd, nc.scalar, nc.sync][qq]
                eng.dma_start(
                    out_v[:, qq * 4:(qq + 1) * 4, :],
                    osb_v[:, qq * 4:(qq + 1) * 4, :],
                )

    nc.compile()
    return nc


def make_in_maps(inputs, adjs, W_w, W_b, a_w, a_b):
    inputs = np.asarray(inputs, dtype=np.float32)
    adjs = np.asarray(adjs)
    W_w = np.asarray(W_w, dtype=np.float32)
    W_b = np.asarray(W_b, dtype=np.float32)
    a_w = np.asarray(a_w, dtype=np.float32)
    ab = float(np.asarray(a_b, dtype=np.float32).reshape(()))

    # xt[p, g, c, n'] = x^T[c*128+p, g*512+n']
    def pack_xt(xb):
        xT = xb.T.astype(np.float16)                      # [256, 2048]
        v = xT.reshape(2, 128, 4, 512)                    # [c, p, g, n']
        return np.ascontiguousarray(
            v.transpose(1, 2, 0, 3).reshape(128, 4096))   # [p, g, c, n']

    # wt[p, c, o] = W_w[o, c*128+p]
    wt = np.ascontiguousarray(
        W_w.T.reshape(2, 128, 128).transpose(1, 0, 2).reshape(128, 256)
    ).astype(np.float16)
    wb = np.ascontiguousarray(W_b.reshape(128, 1)).astype(np.float32)

    # parameter-folded projections
    a1 = a_w[0, :F]
    a2 = a_w[0, F:]
    c1 = a1 @ W_w          # [256]
    c2 = a2 @ W_w          # [256]
    d1 = float(a1 @ W_b)
    d2 = float(a2 @ W_b)
    # cb8[p, c, m] = 0.8*c1[c*128+p] (broadcast over m)
    cb8 = np.ascontiguousarray(
        np.broadcast_to((0.8 * c1).reshape(2, 128, 1).transpose(1, 0, 2),
                        (128, 2, 128)).reshape(128, 256)
    ).astype(np.float16)
    c2p = np.ascontiguousarray(
        c2.reshape(2, 128).T
    ).astype(np.float16)   # [p, c]
    biasc = np.ascontiguousarray(
        np.broadcast_to(
            np.array([0.8 * d1, d2 + ab, 0.2 * (d2 + ab), 0.0],
                     dtype=np.float32),
            (128, 4),
        )
    )
    idf16 = np.eye(128, dtype=np.float16)

    maps = []
    for b in range(BS):
        adjT = adjs[b].T
        maps.append({
            "xt": pack_xt(inputs[b]),
            "adjf": np.ascontiguousarray(adjT[:, :SPLIT]).astype(np.float16),
            "adju": np.ascontiguousarray(adjT[:, SPLIT:]).astype(np.uint8),
            "wt": wt,
            "wb": wb,
            "cb8": cb8,
            "c2": c2p,
            "biasc": biasc,
            "idf16": idf16,
        })
    return maps


def kernel(inputs, adjs, W_w, W_b, a_w, a_b):
    from concourse.bass_utils import run_bass_kernel_spmd

    if "nc" not in _cache:
        _cache["nc"] = _build()
    nc = _cache["nc"]

    in_maps = make_in_maps(inputs, adjs, W_w, W_b, a_w, a_b)
    try:
        res = run_bass_kernel_spmd(nc, in_maps, core_ids=list(range(BS)))
    except Exception:
        # transient NRT_EXEC_UNIT_UNRECOVERABLE etc. -- retry once
        res = run_bass_kernel_spmd(nc, in_maps, core_ids=list(range(BS)))
    out = np.stack([res.results[b]["out"] for b in range(BS)], axis=0)
    return out.astype(np.float32)


# revision 26
# speedup vs baseline: 1.2675x; 1.0210x over previous
"""GAT layer (nn_GATLayer_28106265985525) on 8 Trainium2 NeuronCores.

Batch-parallel: core b computes graph b (bs=8). Scores are built in
transposed [j, i] layout so no per-block PE transposes are needed.

Key algebra: softmax_j is invariant to any positive per-row (i) factor, and
exp(lrelu(s)) == max(e^s, e^{0.2 s}).  With s = fs_i + fd_j + ab:
  E[i,j]            = adj * max(e^s, e^{0.2s})
  E * e^{-0.2 fs_i} = adj * max(U_i * w_j, q_j)      (row-factor cancels)
where U_i = e^{0.8 fs_i}, w_j = e^{fd_j+ab}, q_j = e^{0.2(fd_j+ab)}.
So the inner loop needs NO exp at all: per 128-j block only
  t1 = (U * w_j) max q_j     -- one 4x-mode TensorScalar on DVE (~0.6us)
  M  = t1 * adjT             -- mask multiply, split DVE (f16 mask, 2x
                                TensorTensor) / Pool gpsimd (u8 mask)
  out_ps[u] += M[:,u]^T @ nodesE ; Z[u] += M[:,u]^T @ 1   -- PE
U is a [128, N] broadcast tensor built once in setup; w/q are per-partition
pointer scalars.  fs/fd come from parameter-folded projections
(c1 = a1 @ W etc.) so they are computed straight from x^T.  Final
normalization multiplies by 1/Z (reciprocal over [128,4] quarters).
PSUM note: matmul start=True clears has_written bits for the whole bank,
so only the first accumulation chain per bank clears; later chunks rely on
"overwrite where bit unset" first-write semantics.
"""

import numpy as np
from contextlib import ExitStack

N = 2048
FIN = 256
F = 128
BS = 8
TB = N // 128  # 16 row blocks
SPLIT = 832    # mask-multiply columns on DVE (f16 mask); rest on Pool (u8)

_cache = {}


def _build(reps=1):
    import concourse.bass as bass
    import concourse.tile as tile
    from concourse import mybir, bacc, library_config

    f32, f16, u8 = mybir.dt.float32, mybir.dt.float16, mybir.dt.uint8
    A = mybir.AluOpType
    AF = mybir.ActivationFunctionType

    nc = bacc.Bacc("TRN2", target_bir_lowering=False, debug=False)
    xt_d = nc.declare_dram_parameter("xt", [128, 4096], f16, isOutput=False)
    adjf_d = nc.declare_dram_parameter("adjf", [N, SPLIT], f16, isOutput=False)
    adju_d = nc.declare_dram_parameter("adju", [N, N - SPLIT], u8, isOutput=False)
    wt_d = nc.declare_dram_parameter("wt", [128, FIN], f16, isOutput=False)
    wb_d = nc.declare_dram_parameter("wb", [128, 1], f32, isOutput=False)
    cb8_d = nc.declare_dram_parameter("cb8", [128, FIN], f16, isOutput=False)
    c2_d = nc.declare_dram_parameter("c2", [128, 2], f16, isOutput=False)
    bias_d = nc.declare_dram_parameter("biasc", [128, 4], f32, isOutput=False)
    idf16_d = nc.declare_dram_parameter("idf16", [128, 128], f16, isOutput=False)
    out_d = nc.declare_dram_parameter("out", [N, F], f32, isOutput=True)

    with tile.TileContext(nc) as tc, ExitStack() as ctx:
        consts = ctx.enter_context(tc.tile_pool(name="consts", bufs=1))
        adjp = ctx.enter_context(tc.tile_pool(name="adjp", bufs=6))
        t1p = ctx.enter_context(tc.tile_pool(name="t1p", bufs=3))
        mp = ctx.enter_context(tc.tile_pool(name="mp", bufs=6))
        ps_out = ctx.enter_context(tc.tile_pool(name="ps_out", bufs=1, space="PSUM"))
        ps_set = ctx.enter_context(tc.tile_pool(name="ps_set", bufs=2, space="PSUM"))
        ps_e = ctx.enter_context(tc.tile_pool(name="ps_e", bufs=1, space="PSUM"))

        # gpsimd standard library provides InstTensorTensor (mask multiply)
        nc.gpsimd.load_library(library_config.standard)

        warm_in = consts.tile([128, 1], f32)
        nc.vector.memset(warm_in[:], 0.0)
        ones_col = consts.tile([128, 1], f16)
        nc.vector.memset(ones_col[:], 1.0)

        # ---- consts: order matters (SP in-order; earliest needed first) ----
        cb8_sb = consts.tile([128, FIN], f16)     # [p, (c m)]
        nc.sync.dma_start(cb8_sb[:], cb8_d[:, :])
        bias_sb = consts.tile([128, 4], f32)
        nc.sync.dma_start(bias_sb[:], bias_d[:, :])
        c2_sb = consts.tile([128, 2], f16)
        nc.sync.dma_start(c2_sb[:], c2_d[:, :])
        wt_sb = consts.tile([128, FIN], f16)      # [p, (c o)]
        nc.sync.dma_start(wt_sb[:], wt_d[:, :])

        wt_v = wt_sb[:].rearrange("p (c o) -> p c o", c=2)
        cb8_v = cb8_sb[:].rearrange("p (c m) -> p c m", c=2)

        # ---- xt in 4 chunks: [p, g, c, 512]; xt0 on ACT, rest on Pool ----
        xt_sb = consts.tile([128, 4096], f16)
        xt_v = xt_sb[:].rearrange("p (g c n) -> p g c n", g=4, c=2)
        for g in range(4):
            eng = [nc.scalar, nc.gpsimd, nc.gpsimd, nc.gpsimd][g]
            eng.dma_start(
                xt_sb[:, g * 1024:(g + 1) * 1024], xt_d[:, g * 1024:(g + 1) * 1024]
            )
        # warm the Exp table while xt streams (memset input, no DMA dep)
        warm = consts.tile([128, 1], f32)
        nc.scalar.activation(warm[:], warm_in[:], AF.Exp)

        # ---- persistent PSUM ----
        out_b = [
            ps_out.tile([128, 512], f32, name=f"ob{k}") for k in range(4)
        ]
        misc_ps = ps_out.tile([128, 32], f32)  # cols 0:16 Z, 16:32 fd

        nT16 = consts.tile([128, N], f16)   # nodes^T [o, n]
        U = consts.tile([128, N], f16)      # e^{0.8 fs_i}, broadcast rows
        wq = consts.tile([128, 32], f32)    # cols 0:16 w, 16:32 q
        nE = consts.tile([128, TB * 128], f16)  # nodes [j, o] per block
        nE_v = nE[:].rearrange("p (t o) -> p t o", o=128)

        # ---- fd first on PE (tiny): misc_ps[:, 16+t] = c2 . xT[:, block t] ----
        for t in range(TB):
            g, sub = t // 4, t % 4
            for c in range(2):
                nc.tensor.matmul(
                    misc_ps[:, 16 + t:17 + t],
                    xt_v[:, g, c, sub * 128:(sub + 1) * 128],
                    c2_sb[:, c:c + 1],
                    start=(c == 0), stop=(c == 1),
                )
        nc.scalar.activation(
            wq[:, 0:16], misc_ps[:, 16:32], AF.Exp,
            bias=bias_sb[:, 1:2], scale=1.0,
        )
        nc.scalar.activation(
            wq[:, 16:32], misc_ps[:, 16:32], AF.Exp,
            bias=bias_sb[:, 2:3], scale=0.2,
        )

        # ---- U path (gates the W1 stream) ----
        for g in range(4):
            u_ps = ps_set.tile([128, 512], f32, tag="s")
            for c in range(2):
                nc.tensor.matmul(
                    u_ps[:], cb8_v[:, c, :], xt_v[:, g, c, :],
                    start=(c == 0), stop=(c == 1),
                )
            nc.scalar.activation(
                U[:, g * 512:(g + 1) * 512], u_ps[:],
                AF.Exp, bias=bias_sb[:, 0:1], scale=1.0,
            )

        # ---- adj DMAs for t=0 + remaining small consts on SP ----
        adj_tiles = []
        for t in range(2):
            adjf_t = adjp.tile([128, SPLIT], f16, tag="adjf")
            nc.sync.dma_start(adjf_t[:], adjf_d[t * 128:(t + 1) * 128, :])
            adju_t = adjp.tile([128, N - SPLIT], u8, tag="adju")
            nc.sync.dma_start(adju_t[:], adju_d[t * 128:(t + 1) * 128, :])
            adj_tiles.append((adjf_t, adju_t))
        wb_sb = consts.tile([128, 1], f32)
        nc.sync.dma_start(wb_sb[:], wb_d[:, :])
        idf16 = consts.tile([128, 128], f16)
        nc.sync.dma_start(idf16[:], idf16_d[:, :])

        # ---- nodes^T = W @ x^T + b; nE via PE transpose, interleaved ----
        for g in range(4):
            n_ps = ps_set.tile([128, 512], f32, tag="s")
            for c in range(2):
                nc.tensor.matmul(
                    n_ps[:], wt_v[:, c, :], xt_v[:, g, c, :],
                    start=(c == 0), stop=(c == 1),
                )
            nc.scalar.activation(
                nT16[:, g * 512:(g + 1) * 512], n_ps[:],
                AF.Identity, bias=wb_sb[:], scale=1.0,
            )
            e_ps = ps_e.tile([128, 512], f16, tag="e")
            for k in range(4):
                t = g * 4 + k
                nc.tensor.transpose(
                    e_ps[:, k * 128:(k + 1) * 128],
                    nT16[:, t * 128:(t + 1) * 128],
                    idf16[:],
                )
            nc.scalar.activation(
                nE[:, g * 512:(g + 1) * 512], e_ps[:], AF.Identity
            )

        # ---- main loop ----
        HALF = 1024
        PRO = 3  # blocks whose W1 is split so it can start on half-ready U
        for rep in range(reps):
            t1_tiles = {}

            def w1a(t):
                t1 = t1p.tile([128, N], f16, tag="t1", name=f"t1_{t}")
                t1_tiles[t] = t1
                nc.vector.tensor_scalar(
                    t1[:, 0:HALF], U[:, 0:HALF],
                    wq[:, t:t + 1], wq[:, 16 + t:17 + t], A.mult, A.max,
                )

            def w1b(t):
                t1 = t1_tiles[t]
                nc.vector.tensor_scalar(
                    t1[:, HALF:N], U[:, HALF:N],
                    wq[:, t:t + 1], wq[:, 16 + t:17 + t], A.mult, A.max,
                )

            def w2a(t):
                M = mp.tile([128, N], f16, tag="M", name=f"M_{t}")
                t1_tiles[(t, "M")] = M
                nc.vector.tensor_tensor(
                    M[:, 0:SPLIT], t1_tiles[t][:, 0:SPLIT],
                    adj_tiles[t][0][:], A.mult,
                )

            def w2b_mms(t):
                M = t1_tiles[(t, "M")]
                nc.gpsimd.tensor_tensor(
                    M[:, SPLIT:N], t1_tiles[t][:, SPLIT:N],
                    adj_tiles[t][1][:], A.mult,
                )
                for u in range(TB):
                    # start=True clears has_written bits for the WHOLE bank:
                    # only the first chunk per bank clears; later chunks rely
                    # on first-write (bit unset -> overwrite) semantics.
                    nc.tensor.matmul(
                        out_b[u // 4][:, (u % 4) * 128:(u % 4 + 1) * 128],
                        M[:, u * 128:(u + 1) * 128],
                        nE_v[:, t, :],
                        start=(t == 0 and u % 4 == 0), stop=(t == TB - 1),
                    )
                    nc.tensor.matmul(
                        misc_ps[:, u:u + 1],
                        M[:, u * 128:(u + 1) * 128],
                        ones_col[:],
                        start=(t == 0 and u == 0), stop=(t == TB - 1),
                    )

            def fetch_adj(t):
                if t < len(adj_tiles) or t >= TB:
                    return
                adjf_t = adjp.tile([128, SPLIT], f16, tag="adjf",
                                   name=f"adjf_{t}")
                nc.sync.dma_start(adjf_t[:], adjf_d[t * 128:(t + 1) * 128, :])
                adju_t = adjp.tile([128, N - SPLIT], u8, tag="adju",
                                   name=f"adju_{t}")
                nc.sync.dma_start(adju_t[:], adju_d[t * 128:(t + 1) * 128, :])
                adj_tiles.append((adjf_t, adju_t))

            # prologue: W1 halves so work starts as soon as U[0:1024] exists
            for t in range(PRO):
                fetch_adj(t)
                w1a(t)
                w2a(t)
            for t in range(PRO):
                w1b(t)
                w2b_mms(t)
                fetch_adj(PRO + t)
            for t in range(PRO, TB):
                fetch_adj(t + PRO)
                t1 = t1p.tile([128, N], f16, tag="t1", name=f"t1_{t}")
                t1_tiles[t] = t1
                nc.vector.tensor_scalar(
                    t1[:], U[:], wq[:, t:t + 1], wq[:, 16 + t:17 + t],
                    A.mult, A.max,
                )
                w2a(t)
                w2b_mms(t)

            # ---- normalize (all norms first), then write out ----
            rc = consts.tile([128, 16], f32, tag="rc")
            osb = consts.tile([128, TB * 128], f32, tag="osb")
            out_v = out_d[:, :].rearrange("(t p) o -> p t o", p=128)
            osb_v = osb[:].rearrange("p (t o) -> p t o", o=128)
            for qq in range(4):
                nc.vector.reciprocal(
                    rc[:, qq * 4:(qq + 1) * 4], misc_ps[:, qq * 4:(qq + 1) * 4]
                )
                for u in range(qq * 4, qq * 4 + 4):
                    src = out_b[u // 4][:, (u % 4) * 128:(u % 4 + 1) * 128]
                    dst = osb[:, u * 128:(u + 1) * 128]
                    if u % 2 == 0:
                        nc.vector.tensor_scalar(
                            dst, src, rc[:, u:u + 1], None, A.mult
                        )
                    else:
                        nc.scalar.activation(
                            dst, src, AF.Copy, bias=0.0, scale=rc[:, u:u + 1]
                        )
                eng = [nc.sync, nc.gpsimd, nc.sync, nc.gpsimd][qq]
                eng.dma_start(
                    out_v[:, qq * 4:(qq + 1) * 4, :],
                    osb_v[:, qq * 4:(qq + 1) * 4, :],
                )

    nc.compile()
    return nc


def make_in_maps(inputs, adjs, W_w, W_b, a_w, a_b):
    inputs = np.asarray(inputs, dtype=np.float32)
    adjs = np.asarray(adjs)
    W_w = np.asarray(W_w, dtype=np.float32)
    W_b = np.asarray(W_b, dtype=np.float32)
    a_w = np.asarray(a_w, dtype=np.float32)
    ab = float(np.asarray(a_b, dtype=np.float32).reshape(()))

    # xt[p, g, c, n'] = x^T[c*128+p, g*512+n']
    def pack_xt(xb):
        xT = xb.T.astype(np.float16)                      # [256, 2048]
        v = xT.reshape(2, 128, 4, 512)                    # [c, p, g, n']
        return np.ascontiguousarray(
            v.transpose(1, 2, 0, 3).reshape(128, 4096))   # [p, g, c, n']

    # wt[p, c, o] = W_w[o, c*128+p]
    wt = np.ascontiguousarray(
        W_w.T.reshape(2, 128, 128).transpose(1, 0, 2).reshape(128, 256)
    ).astype(np.float16)
    wb = np.ascontiguousarray(W_b.reshape(128, 1)).astype(np.float32)

    # parameter-folded projections
    a1 = a_w[0, :F]
    a2 = a_w[0, F:]
    c1 = a1 @ W_w          # [256]
    c2 = a2 @ W_w          # [256]
    d1 = float(a1 @ W_b)
    d2 = float(a2 @ W_b)
    # cb8[p, c, m] = 0.8*c1[c*128+p] (broadcast over m)
    cb8 = np.ascontiguousarray(
        np.broadcast_to((0.8 * c1).reshape(2, 128, 1).transpose(1, 0, 2),
                        (128, 2, 128)).reshape(128, 256)
    ).astype(np.float16)
    c2p = np.ascontiguousarray(
        c2.reshape(2, 128).T
    ).astype(np.float16)   # [p, c]
    biasc = np.ascontiguousarray(
        np.broadcast_to(
            np.array([0.8 * d1, d2 + ab, 0.2 * (d2 + ab), 0.0],
                     dtype=np.float32),
            (128, 4),
        )
    )
    idf16 = np.eye(128, dtype=np.float16)

    maps = []
    for b in range(BS):
        adjT = adjs[b].T
        maps.append({
            "xt": pack_xt(inputs[b]),
            "adjf": np.ascontiguousarray(adjT[:, :SPLIT]).astype(np.float16),
            "adju": np.ascontiguousarray(adjT[:, SPLIT:]).astype(np.uint8),
            "wt": wt,
            "wb": wb,
            "cb8": cb8,
            "c2": c2p,
            "biasc": biasc,
            "idf16": idf16,
        })
    return maps


def kernel(inputs, adjs, W_w, W_b, a_w, a_b):
    from concourse.bass_utils import run_bass_kernel_spmd

    if "nc" not in _cache:
        _cache["nc"] = _build()
    nc = _cache["nc"]

    in_maps = make_in_maps(inputs, adjs, W_w, W_b, a_w, a_b)
    try:
        res = run_bass_kernel_spmd(nc, in_maps, core_ids=list(range(BS)))
    except Exception:
        # transient NRT_EXEC_UNIT_UNRECOVERABLE etc. -- retry once
        res = run_bass_kernel_spmd(nc, in_maps, core_ids=list(range(BS)))
    out = np.stack([res.results[b]["out"] for b in range(BS)], axis=0)
    return out.astype(np.float32)
